# revision 1
# baseline (speedup 1.0000x reference)
"""Trainium2 Bass kernel for nn_ConditionalDisCoLoss.

loss = BCEWithLogits(inputs, targets)
     + dCor_masked(sigmoid(inputs), spectators, mask=spectators>=0.5)

Reformulation (no centered n x n matrices):
  p = sigmoid(x), m = (s >= 0.5), c = max(sum m, 1)
  A_i = sum_j m_i m_j |p_i - p_j|,  B_i likewise for s
  Sxy = sum_ij m_i m_j |p_i-p_j||s_i-s_j|
  Sxx = 2c*sum(m p^2) - 2(sum m p)^2   (closed form), Syy likewise
  Vxy = Sxy - (2/c) sum A_i B_i + (sum A)(sum B)/c^2  (and Vxx, Vyy)
  dcor = sqrt(max(Vxy,eps'))/sqrt(...)   with the reference's eps placement

Distribution + symmetry: the pair matrix is symmetric, so only j-bands
at or above each row's band are computed.  Global i-tiles (128 rows) are
dealt round-robin: core k owns i-tiles {8*it + k}, whose band is exactly
`it`, so every core runs the SAME program (jt in [it, 8)) on different
gathered row data - 36 of 64 tiles each.

Per tile [128 x 1024]:
 - PE: masked pairwise diffs D1 = m_i m_j (p_i - p_j) via K=4 bf16
   matmuls (bf16 hi+lo split of p keeps ~1e-7 element accuracy)
 - ACT: U = |D1| (bf16) + fused row-sum accum (A row-part); part of |D2|
 - DVE: rest of |D2| via abs_max + fused accum; product U*V with fused
   row-sum accum (Sxy partials)
 - PE: column sums of U,V for strictly-upper tiles (the transposed
   pairs' row sums) via [128,128]^T @ ones accumulated in one PSUM bank
Host combines per-core partial A/B vectors and scalars in float64.
"""

import numpy as np
from contextlib import ExitStack

import concourse.bass as bass
import concourse.bacc as bacc
import concourse.tile as tile
from concourse import mybir
from concourse.bass_utils import run_bass_kernel_spmd

N = 8192
NCORES = 8
STRIP = N // NCORES      # 1024 rows per core (gathered, not contiguous)
P = 128
JT = 1024                # j-tile width (one band = one j-tile)
NB = N // JT             # 8 bands
NIT = STRIP // P         # 8 i-tiles per core; i-tile it sits in band it
F_FULL = N // P          # 64
F_STRIP = STRIP // P     # 8
SPLIT_ACT = 704          # columns of |D2| done on ACT; rest on DVE

F32 = mybir.dt.float32
BF16 = mybir.dt.bfloat16
F32R = mybir.dt.float32r
ALU = mybir.AluOpType
ACTF = mybir.ActivationFunctionType
AX = mybir.AxisListType

NOUT = 16
# partials slots: 5 sum(R_diag), 6 sum(m), 7 sum(m*p), 8 sum(m*p^2),
#                 9 sum(m*s), 10 sum(m*s^2), 11 sum(bce), 12 sum(R_upper)
NCOLP = 112  # colparts: 7 bands x (8 quarters A | 8 quarters B)


def _build():
    nc = bacc.Bacc("TRN2", target_bir_lowering=False, debug=False,
                   num_devices=NCORES, enable_asserts=False)

    x_full = nc.dram_tensor("x_full", [N, 1], F32, kind="ExternalInput")
    s_full = nc.dram_tensor("s_full", [N], F32, kind="ExternalInput")
    x_strip = nc.dram_tensor("x_strip", [STRIP, 1], F32, kind="ExternalInput")
    t_strip = nc.dram_tensor("t_strip", [STRIP, 1], F32, kind="ExternalInput")
    s_strip = nc.dram_tensor("s_strip", [STRIP], F32, kind="ExternalInput")
    out = nc.dram_tensor("partials", [NOUT], F32, kind="ExternalOutput")
    rowp = nc.dram_tensor("rowparts", [P, 16], F32, kind="ExternalOutput")
    colp = nc.dram_tensor("colparts", [P, NCOLP], F32, kind="ExternalOutput")

    with tile.TileContext(nc) as tc, ExitStack() as ctx:
        pre = ctx.enter_context(tc.tile_pool(name="pre", bufs=1))
        uvp = ctx.enter_context(tc.tile_pool(name="uv", bufs=3))
        accp = ctx.enter_context(tc.tile_pool(name="acc", bufs=2))
        psp = ctx.enter_context(tc.tile_pool(name="psp", bufs=3, space="PSUM"))
        psc = ctx.enter_context(tc.tile_pool(name="psc", bufs=1, space="PSUM"))

        # ---------- preprocessing: full vectors -> moving operands ----------
        xf = pre.tile([P, F_FULL], F32)
        sf = pre.tile([P, F_FULL], F32)
        nc.sync.dma_start(out=xf, in_=x_full.ap().rearrange("(p f) one -> p (f one)", p=P))
        nc.scalar.dma_start(out=sf, in_=s_full.ap().rearrange("(p f) -> p f", p=P))

        pf = pre.tile([P, F_FULL], F32)
        nc.scalar.activation(pf, xf, ACTF.Sigmoid)
        mf = pre.tile([P, F_FULL], F32)
        nc.vector.tensor_scalar(mf, sf, 0.5, None, ALU.is_ge)
        af = pre.tile([P, F_FULL], F32)
        nc.vector.tensor_tensor(out=af, in0=mf, in1=pf, op=ALU.mult)
        cf = pre.tile([P, F_FULL], F32)
        nc.vector.tensor_tensor(out=cf, in0=mf, in1=sf, op=ALU.mult)

        # moving operands (f32, fed to the PE as float32r via bitcast):
        # RA rows: m, a=m*p   RB rows: m, c=m*s
        RA = pre.tile([2, N], F32)
        RB = pre.tile([2, N], F32)
        for eng, dst, row, src in ((nc.sync, RA, 0, mf), (nc.scalar, RA, 1, af),
                                   (nc.sync, RB, 0, mf), (nc.scalar, RB, 1, cf)):
            eng.dma_start(out=dst[row:row + 1, :], in_=src)

        # ---------- preprocessing: gathered strip -> stationary operands ----------
        # [16, 64] layout: strip position s = p*64 + f (DMA-friendly 256B rows)
        PS, FS = 16, 64
        xs = pre.tile([PS, FS], F32)
        ts = pre.tile([PS, FS], F32)
        ss = pre.tile([PS, FS], F32)
        nc.sync.dma_start(out=xs, in_=x_strip.ap().rearrange("(p f) one -> p (f one)", p=PS))
        nc.scalar.dma_start(out=ts, in_=t_strip.ap().rearrange("(p f) one -> p (f one)", p=PS))
        nc.sync.dma_start(out=ss, in_=s_strip.ap().rearrange("(p f) -> p f", p=PS))

        ps_ = pre.tile([PS, FS], F32)
        nc.scalar.activation(ps_, xs, ACTF.Sigmoid)
        ms = pre.tile([PS, FS], F32)
        nc.vector.tensor_scalar(ms, ss, 0.5, None, ALU.is_ge)
        negm = pre.tile([PS, FS], F32)
        nc.vector.tensor_scalar(negm, ms, -1.0, None, ALU.mult)

        bs = pre.tile([PS, FS], F32)
        nc.vector.tensor_tensor(out=bs, in0=ms, in1=ps_, op=ALU.mult)
        ds = pre.tile([PS, FS], F32)
        nc.vector.tensor_tensor(out=ds, in0=ms, in1=ss, op=ALU.mult)

        # stationary operands: LA rows (b, -m), LB rows (d, -m)
        LA = pre.tile([2, STRIP], F32)
        LB = pre.tile([2, STRIP], F32)
        for eng, dst, row, src in ((nc.sync, LA, 0, bs), (nc.scalar, LA, 1, negm),
                                   (nc.sync, LB, 0, ds), (nc.scalar, LB, 1, negm)):
            eng.dma_start(out=dst[row:row + 1, :], in_=src)

        # ---------- O(n) scalar columns (strip tiles live on partitions 0:16,
        # rest of cat stays zero and drops out of the final ones-matmul) ----------
        cat = pre.tile([P, NOUT], F32)
        nc.vector.memset(cat, 0.0)
        junk_s = pre.tile([PS, FS], F32)

        nc.vector.tensor_reduce(cat[0:PS, 6:7], ms, AX.X, ALU.add)
        nc.vector.tensor_reduce(cat[0:PS, 7:8], bs, AX.X, ALU.add)
        nc.vector.scalar_tensor_tensor(out=junk_s, in0=bs, scalar=0.0,
                                       in1=ps_, op0=ALU.bypass, op1=ALU.mult,
                                       accum_out=cat[0:PS, 8:9])
        nc.vector.tensor_reduce(cat[0:PS, 9:10], ds, AX.X, ALU.add)
        junk_s2 = pre.tile([PS, FS], F32)
        nc.vector.scalar_tensor_tensor(out=junk_s2, in0=ds, scalar=0.0,
                                       in1=ss, op0=ALU.bypass, op1=ALU.mult,
                                       accum_out=cat[0:PS, 10:11])

        # ---------- main pass: tiles (it, jt) with jt >= it ----------
        ones = pre.tile([P, 1], BF16)
        nc.vector.memset(ones, 1.0)
        onesf = pre.tile([P, 1], F32)
        nc.vector.memset(onesf, 1.0)

        # per-tile column sums, rectangular [it][jt][16] layout (no PSUM
        # accumulation -- scheduler may reorder same-engine matmuls, so
        # every tile writes its own fresh column; reduced over it at the end)
        colacc = psc.tile([P, NIT, NB, 16], F32)

        AA = pre.tile([P, NIT], F32)
        BB = pre.tile([P, NIT], F32)
        RRd = pre.tile([P, NIT], F32)
        RRu = pre.tile([P, NIT], F32)
        nc.vector.memset(RRu, 0.0)

        for it in range(NIT):
            njt = NB - it
            Ap = accp.tile([P, NB], F32, tag="Ap")
            Bp = accp.tile([P, 2 * NB], F32, tag="Bp")
            Rp = accp.tile([P, NB], F32, tag="Rp")
            lA = LA[:, it * P:(it + 1) * P]
            lB = LB[:, it * P:(it + 1) * P]
            for jj in range(njt):
                jt = it + jj
                psA = psp.tile([P, JT], F32, tag="ps")
                psB = psp.tile([P, JT], F32, tag="ps")
                for h in range(JT // 512):
                    j0 = jt * JT + h * 512
                    nc.tensor.matmul(psA[:, h * 512:(h + 1) * 512],
                                     lhsT=lA.bitcast(F32R),
                                     rhs=RA[:, j0:j0 + 512].bitcast(F32R),
                                     start=True, stop=True)
                    nc.tensor.matmul(psB[:, h * 512:(h + 1) * 512],
                                     lhsT=lB.bitcast(F32R),
                                     rhs=RB[:, j0:j0 + 512].bitcast(F32R),
                                     start=True, stop=True)
                U = uvp.tile([P, JT], BF16, tag="U")
                V = uvp.tile([P, JT], F32, tag="V")
                nc.scalar.activation(U, psA, ACTF.Abs, accum_out=Ap[:, jj:jj + 1])
                nc.scalar.activation(V[:, 0:SPLIT_ACT], psB[:, 0:SPLIT_ACT], ACTF.Abs,
                                     accum_out=Bp[:, 2 * jj:2 * jj + 1])
                # |x| on DVE in 2 ops (only one PSUM operand allowed per op):
                # Vn = -psB_slice (PSUM->SBUF), then V2 = max(Vn, psB_slice)
                Vn = uvp.tile([P, JT - SPLIT_ACT], F32, tag="Vn")
                nc.vector.tensor_scalar(Vn, psB[:, SPLIT_ACT:JT], -1.0, None, ALU.mult)
                nc.vector.scalar_tensor_tensor(out=V[:, SPLIT_ACT:JT],
                                               in0=Vn, scalar=0.0,
                                               in1=psB[:, SPLIT_ACT:JT],
                                               op0=ALU.bypass, op1=ALU.max,
                                               accum_out=Bp[:, 2 * jj + 1:2 * jj + 2])
                W = uvp.tile([P, JT], F32, tag="W")
                nc.vector.scalar_tensor_tensor(out=W, in0=U, scalar=0.0,
                                               in1=V, op0=ALU.bypass, op1=ALU.mult,
                                               accum_out=Rp[:, jj:jj + 1])
                if jt > it:
                    # transposed pairs' row sums = column sums, via PE
                    for q in range(8):
                        nc.tensor.matmul(colacc[:, it, jt, q:q + 1],
                                         lhsT=U[:, q * P:(q + 1) * P], rhs=ones,
                                         start=True, stop=True)
                        nc.tensor.matmul(colacc[:, it, jt, q + 8:q + 9],
                                         lhsT=V[:, q * P:(q + 1) * P], rhs=onesf,
                                         start=True, stop=True)
            nc.vector.tensor_reduce(AA[:, it:it + 1], Ap[:, 0:njt], AX.X, ALU.add)
            nc.vector.tensor_reduce(BB[:, it:it + 1], Bp[:, 0:2 * njt], AX.X, ALU.add)
            nc.vector.tensor_copy(RRd[:, it:it + 1], Rp[:, 0:1])
            if njt > 1:
                nc.vector.tensor_reduce(RRu[:, it:it + 1], Rp[:, 1:njt], AX.X, ALU.add)

        # ---------- outputs ----------
        # BCE partial: relu(x) - x*t + softplus(-|x|) = relu - xt + ln(1+exp(-|x|))
        rx = pre.tile([PS, FS], F32)
        nc.vector.tensor_scalar(rx, xs, 0.0, None, ALU.max)
        xt = pre.tile([PS, FS], F32)
        nc.vector.tensor_tensor(out=xt, in0=xs, in1=ts, op=ALU.mult)
        axx = pre.tile([PS, FS], F32)
        nc.scalar.activation(axx, xs, ACTF.Abs)
        enx = pre.tile([PS, FS], F32)
        nc.scalar.activation(enx, axx, ACTF.Exp, scale=-1.0)
        sp = pre.tile([PS, FS], F32)
        nc.scalar.activation(sp, enx, ACTF.Ln, bias=1.0)
        t1 = pre.tile([PS, FS], F32)
        nc.vector.tensor_tensor(out=t1, in0=rx, in1=xt, op=ALU.subtract)
        t2 = pre.tile([PS, FS], F32)
        nc.vector.scalar_tensor_tensor(out=t2, in0=t1, scalar=0.0, in1=sp,
                                       op0=ALU.add, op1=ALU.add,
                                       accum_out=cat[0:PS, 11:12])

        nc.vector.tensor_reduce(cat[:, 5:6], RRd, AX.X, ALU.add)
        nc.vector.tensor_reduce(cat[:, 12:13], RRu, AX.X, ALU.add)

        pcat = psp.tile([NOUT, 1], F32, tag="ps")
        nc.tensor.matmul(pcat, lhsT=cat, rhs=onesf, start=True, stop=True)
        outt = pre.tile([NOUT, 1], F32)
        nc.scalar.copy(outt, pcat)
        nc.sync.dma_start(out=out.ap().rearrange("(a b) -> a b", b=1), in_=outt)

        rowt = pre.tile([P, 16], F32)
        nc.vector.tensor_copy(rowt[:, 0:8], AA)
        nc.vector.tensor_copy(rowt[:, 8:16], BB)
        nc.sync.dma_start(out=rowp.ap(), in_=rowt)

        # reduce per-tile column sums over it (strided AP: last dim = it)
        colt = pre.tile([P, NCOLP], F32)
        for jt in range(1, NB):
            for half in range(2):  # 0: A quarters, 1: B quarters
                src = colacc[:, 0:jt, jt, half * 8:(half + 1) * 8]
                src = src.rearrange("p i q -> p q i")
                nc.vector.tensor_reduce(
                    colt[:, (jt - 1) * 16 + half * 8:(jt - 1) * 16 + (half + 1) * 8],
                    src, AX.X, ALU.add)
        nc.scalar.dma_start(out=colp.ap(), in_=colt)

    nc.compile()
    return nc


_NC_CACHE = None


def _get_nc():
    global _NC_CACHE
    if _NC_CACHE is None:
        _NC_CACHE = _build()
    return _NC_CACHE


def _row_index(k):
    """Global row indices owned by core k (i-tiles 8*it + k)."""
    idx = []
    for it_ in range(NIT):
        t = 8 * it_ + k
        idx.append(np.arange(t * P, (t + 1) * P))
    return np.concatenate(idx)


def _make_in_maps(inputs, targets, spectators):
    x = np.ascontiguousarray(np.asarray(inputs, dtype=np.float32)).reshape(N, 1)
    t = np.ascontiguousarray(np.asarray(targets, dtype=np.float32)).reshape(N, 1)
    s = np.ascontiguousarray(np.asarray(spectators, dtype=np.float32)).reshape(N)
    in_maps = []
    for k in range(NCORES):
        idx = _row_index(k)
        in_maps.append({
            "x_full": x,
            "s_full": s,
            "x_strip": np.ascontiguousarray(x[idx]),
            "t_strip": np.ascontiguousarray(t[idx]),
            "s_strip": np.ascontiguousarray(s[idx]),
        })
    return in_maps


def _combine(results):
    """results: list of per-core dicts with partials/rowparts/colparts."""
    g = np.zeros(NOUT, np.float64)
    A = np.zeros(N, np.float64)
    B = np.zeros(N, np.float64)
    for k in range(NCORES):
        g += results[k]["partials"].astype(np.float64)
        rowpart = results[k]["rowparts"].astype(np.float64)  # [128, 16]
        idx = _row_index(k)
        A[idx] += rowpart[:, 0:8].T.reshape(-1)
        B[idx] += rowpart[:, 8:16].T.reshape(-1)
        colpart = results[k]["colparts"].astype(np.float64)  # [128, 7*16]
        cp = colpart.reshape(P, 7, 16)
        # col index (jt-1)*16 + q (A) / + 8 + q (B); j = jt*1024 + q*128 + p
        Ac = cp[:, :, 0:8].transpose(1, 2, 0).reshape(-1)   # [7*8*128] j-ordered
        Bc = cp[:, :, 8:16].transpose(1, 2, 0).reshape(-1)
        A[JT:] += Ac
        B[JT:] += Bc

    cnt, smp, smp2, sms, sms2, bce_sum = g[6], g[7], g[8], g[9], g[10], g[11]
    Sxy = g[5] + 2.0 * g[12]
    sAB = float(A @ B)
    sAA = float(A @ A)
    sBB = float(B @ B)
    Tx = float(A.sum())
    Ty = float(B.sum())

    bce = bce_sum / N
    c = max(cnt, 1.0)
    Sxx = 2.0 * c * smp2 - 2.0 * smp * smp
    Syy = 2.0 * c * sms2 - 2.0 * sms * sms
    Vxy = Sxy - (2.0 / c) * sAB + Tx * Ty / (c * c)
    Vxx = Sxx - (2.0 / c) * sAA + Tx * Tx / (c * c)
    Vyy = Syy - (2.0 / c) * sBB + Ty * Ty / (c * c)
    EPS = 1e-8
    dcov = np.sqrt(max(Vxy / (c * c), EPS))
    dvx = np.sqrt(max(Vxx / (c * c), EPS))
    dvy = np.sqrt(max(Vyy / (c * c), EPS))
    dcor = dcov / (dvx * dvy)
    loss = bce + (dcor if cnt > 0 else 0.0)
    return np.float32(loss)


def kernel(inputs, targets, spectators):
    nc = _get_nc()
    in_maps = _make_in_maps(inputs, targets, spectators)
    res = run_bass_kernel_spmd(nc, in_maps, list(range(NCORES)))
    return _combine(res.results)


if __name__ == "__main__":
    d = np.load("/root/problem/cached_io.npz")
    out = kernel(d["inputs"], d["targets"], d["spectators"])
    exp = float(d["expected"])
    rel = abs(float(out) - exp) / abs(exp)
    print(f"kernel: {float(out):.8f}  expected: {exp:.8f}  rel err: {rel:.3e}")



# revision 15
# speedup vs baseline: 6.3714x; 6.3714x over previous
"""Trainium2 Bass kernel for nn_ConditionalDisCoLoss.

loss = BCEWithLogits(inputs, targets)
     + dCor_masked(sigmoid(inputs), spectators, mask=spectators>=0.5)

Key identities (see _combine):
  With A_i = sum_j m_j|p_i-p_j|, B_i likewise for s, and
  Sxy = sum_ij m_i m_j |p_i-p_j||s_i-s_j|:
    Vxy = Sxy - (2/c) sum A_i B_i + (sum A)(sum B)/c^2   (and Vxx, Vyy)
  A_i, B_i have O(n log n) closed forms via sorting (1-D data), and
  Sxx, Syy have O(n) closed forms, so the ONLY O(n^2) quantity is Sxy.

Device computes Sxy and the BCE partials; host does the O(n) / O(n log n)
filtering, packing and scalar assembly.

Sxy device trick: |a*b| == |a|*|b| exactly in IEEE, and
  D1*D2 = m_i m_j (p_i-p_j)(s_i-s_j)
        = (m_i p_i s_i)*m_j - (m_i p_i)*(m_j s_j) - (m_i s_i)*(m_j p_j)
          + m_i*(m_j p_j s_j)
is a rank-4 bilinear form -> ONE K=4 f32r matmul produces D1*D2 directly
in PSUM; a single abs op with fused row-accumulation per [128 x 1024]
tile (round-robined over ACT/DVE/Pool) yields the Sxy partials.

Distribution: samples with m=1 are host-compacted (c ~ n/2) and padded to
CAP=4096 (pad rows get m=0 and drop out). 32 global row-tiles of 128 are
dealt round-robin: core k owns i-tiles {8t+k}, whose 1024-wide band is t,
so every core runs the SAME program (jt in [it, NB)) - 10 tiles each.
Diagonal-band tiles cover their band block fully (counted once); upper
tiles are doubled in the combine. BCE runs on contiguous 1024-row strips
of the raw inputs. Falls back to a CAP=8192 build if c > 4096.
"""

import numpy as np
from contextlib import ExitStack

import concourse.bass as bass
import concourse.bacc as bacc
import concourse.tile as tile
from concourse import mybir
from concourse.bass_utils import run_bass_kernel_spmd

N = 8192
NCORES = 8
P = 128
JT = 1024
BSTRIP = N // NCORES     # 1024 BCE rows per core
EPS = 1e-8

F32 = mybir.dt.float32
BF16 = mybir.dt.bfloat16
F32R = mybir.dt.float32r
ALU = mybir.AluOpType
ACTF = mybir.ActivationFunctionType
AX = mybir.AxisListType


def _build(cap):
    """cap: padded compacted-sample capacity (multiple of 1024, /8 cores)."""
    nb = cap // JT           # bands == i-tiles per core
    nit = nb
    ntiles = nb * (nb + 1) // 2
    rw = ntiles + 1          # Rp columns: tiles + bce

    nc = bacc.Bacc("TRN2", target_bir_lowering=False, debug=False,
                   num_devices=NCORES, enable_asserts=False)

    rhs_d = nc.dram_tensor("rhs4", [4, cap], F32, kind="ExternalInput")
    lhs_d = nc.dram_tensor("lhsT4", [4, P * nit], F32, kind="ExternalInput")
    xs_d = nc.dram_tensor("x_strip", [BSTRIP], F32, kind="ExternalInput")
    ts_d = nc.dram_tensor("t_strip", [BSTRIP], F32, kind="ExternalInput")
    out_d = nc.dram_tensor("rowout", [P, rw], F32, kind="ExternalOutput")

    with tile.TileContext(nc) as tc, ExitStack() as ctx:
        pre = ctx.enter_context(tc.tile_pool(name="pre", bufs=1))
        uvp = ctx.enter_context(tc.tile_pool(name="uv", bufs=3))
        psp = ctx.enter_context(tc.tile_pool(name="psp", bufs=3, space="PSUM"))

        # act-table warmup: all funcs (Abs/Exp/Ln) live in one set; issuing
        # the first ACT op before any data dependency overlaps the table
        # load with the input DMAs.
        warm = pre.tile([P, 1], F32)
        nc.vector.memset(warm, 0.0)
        warm2 = pre.tile([P, 1], F32)
        nc.scalar.activation(warm2, warm, ACTF.Abs)

        rhs = pre.tile([4, cap], F32R)
        nc.sync.dma_start(out=rhs, in_=rhs_d.ap().bitcast(F32R))
        lhsT = pre.tile([4, P * nit], F32R)
        nc.scalar.dma_start(out=lhsT, in_=lhs_d.ap().bitcast(F32R))
        xs = pre.tile([P, BSTRIP // P], F32)
        nc.sync.dma_start(out=xs, in_=xs_d.ap().rearrange("(p f) -> p f", p=P))
        ts = pre.tile([P, BSTRIP // P], F32)
        nc.scalar.dma_start(out=ts, in_=ts_d.ap().rearrange("(p f) -> p f", p=P))

        Rp = pre.tile([P, rw], F32)

        # ---------- main pass: tiles (it, jt) with jt >= it ----------
        # slot layout: diag tile (it,it) -> col it; upper tiles -> nit+u
        # Pool/gpsimd cannot read PSUM, so the abs ops alternate ACT/DVE
        # (ACT slightly faster -> gets the odd one out).
        engs = (nc.scalar, nc.vector)
        ei = 0
        u = 0
        for it in range(nit):
            lA = lhsT[:, it * P:(it + 1) * P]
            for jt in range(it, nb):
                psT = psp.tile([P, JT], F32, tag="ps")
                for h in range(JT // 512):
                    j0 = jt * JT + h * 512
                    nc.tensor.matmul(psT[:, h * 512:(h + 1) * 512],
                                     lhsT=lA,
                                     rhs=rhs[:, j0:j0 + 512],
                                     start=True, stop=True)
                sl = it if jt == it else nit + u
                if jt > it:
                    u += 1
                W = uvp.tile([P, JT], BF16, tag="W")
                eng = engs[ei % 2]
                ei += 1
                if eng is nc.scalar:
                    nc.scalar.activation(W, psT, ACTF.Abs,
                                         accum_out=Rp[:, sl:sl + 1])
                else:
                    # DVE lane: no 1-op abs from PSUM exists, so accumulate
                    # relu(T) instead; host applies sum|T| = 2*sum relu(T)
                    # - sum T, with sum T in closed form (rank-4 bilinear).
                    eng.tensor_scalar(W, psT, 0.0, 0.0, ALU.max,
                                      ALU.add, accum_out=Rp[:, sl:sl + 1])

        # ---------- BCE partial: relu(x) - x*t + ln(1+exp(-|x|)) ----------
        ax = pre.tile([P, BSTRIP // P], F32)
        nc.vector.scalar_tensor_tensor(out=ax, in0=xs, scalar=-1.0, in1=xs,
                                       op0=ALU.mult, op1=ALU.max)
        enx = pre.tile([P, BSTRIP // P], F32)
        nc.scalar.activation(enx, ax, ACTF.Exp, scale=-1.0)
        sp = pre.tile([P, BSTRIP // P], F32)
        nc.scalar.activation(sp, enx, ACTF.Ln, bias=1.0)
        rx = pre.tile([P, BSTRIP // P], F32)
        nc.vector.tensor_scalar(rx, xs, 0.0, None, ALU.max)
        xt = pre.tile([P, BSTRIP // P], F32)
        nc.vector.tensor_tensor(out=xt, in0=xs, in1=ts, op=ALU.mult)
        t1 = pre.tile([P, BSTRIP // P], F32)
        nc.vector.tensor_tensor(out=t1, in0=rx, in1=xt, op=ALU.subtract)
        t2 = pre.tile([P, BSTRIP // P], F32)
        nc.vector.scalar_tensor_tensor(out=t2, in0=t1, scalar=0.0, in1=sp,
                                       op0=ALU.add, op1=ALU.add,
                                       accum_out=Rp[:, ntiles:ntiles + 1])

        nc.sync.dma_start(out=out_d.ap(), in_=Rp)

    nc.compile()
    return nc


_NC_CACHE = {}


def _get_nc(cap=4096):
    if cap not in _NC_CACHE:
        _NC_CACHE[cap] = _build(cap)
    return _NC_CACHE[cap]


def _make_in_maps(x, t, p, ss, c, cap):
    """x,t: raw [N] f32. p,ss: compacted selected samples (len c) f32."""
    nit = cap // JT
    rhs = np.zeros((4, cap), np.float32)
    rhs[0, :c] = 1.0
    rhs[1, :c] = ss
    rhs[2, :c] = p
    rhs[3, :c] = p * ss
    in_maps = []
    for k in range(NCORES):
        lhsT = np.empty((4, P * nit), np.float32)
        for t_ in range(nit):
            g = 8 * t_ + k
            sl = slice(P * g, P * g + P)
            dl = slice(P * t_, P * t_ + P)
            lhsT[0, dl] = rhs[3, sl]
            lhsT[1, dl] = -rhs[2, sl]
            lhsT[2, dl] = -rhs[1, sl]
            lhsT[3, dl] = rhs[0, sl]
        in_maps.append({
            "rhs4": rhs,
            "lhsT4": np.ascontiguousarray(lhsT),
            "x_strip": np.ascontiguousarray(x[BSTRIP * k:BSTRIP * (k + 1)]),
            "t_strip": np.ascontiguousarray(t[BSTRIP * k:BSTRIP * (k + 1)]),
        })
    return in_maps


def _dist_rowsums(v):
    """A_i = sum_j |v_i - v_j| in O(n log n) (f64)."""
    v = v.astype(np.float64)
    n = len(v)
    order = np.argsort(v, kind="stable")
    sv = v[order]
    cs = np.cumsum(sv)
    S = cs[-1] if n else 0.0
    r = np.arange(1, n + 1)
    Ar = sv * (2 * r - n) - 2.0 * cs + S
    A = np.empty(n)
    A[order] = Ar
    return A


def _tile_layout(nb):
    """Loop-order tile list [(it, jt, slot, is_act)] mirroring _build."""
    nit = nb
    out = []
    ei = 0
    u = 0
    for it in range(nit):
        for jt in range(it, nb):
            sl = it if jt == it else nit + u
            if jt > it:
                u += 1
            out.append((it, jt, sl, ei % 2 == 0))
            ei += 1
    return out


def _combine(results, p, ss, c, cap):
    nb = cap // JT
    nit = nb
    ntiles = nb * (nb + 1) // 2

    # closed-form block sums for the relu->abs correction on DVE tiles:
    # T_ij = sum_r lhsT[r,i] rhs[r,j]  =>  sum_{tile} T = sum_r L_r(k,it) S_r(jt)
    rhs = np.zeros((4, cap))
    rhs[0, :c] = 1.0
    rhs[1, :c] = ss.astype(np.float64)
    rhs[2, :c] = p.astype(np.float64)
    rhs[3, :c] = rhs[1, :c] * rhs[2, :c]
    S = rhs.reshape(4, nb, JT).sum(axis=2)             # [4, nb]
    G = rhs.reshape(4, cap // P, P).sum(axis=2)        # [4, n_chunks] per 128-chunk
    sgn = np.array([1.0, -1.0, -1.0, 1.0])
    layout = _tile_layout(nb)

    Sxy = 0.0
    bce_sum = 0.0
    for k in range(NCORES):
        r = results[k]["rowout"].astype(np.float64)
        for it, jt, sl, is_act in layout:
            v = r[:, sl].sum()
            if not is_act:
                g = 8 * it + k
                # L_r = chunk sums of lhsT rows (mps, -mp, -ms, m)
                L = sgn * G[::-1, g]
                v = 2.0 * v - float(L @ S[:, jt])
            Sxy += v if jt == it else 2.0 * v
        bce_sum += r[:, ntiles].sum()
    bce = bce_sum / N
    if c == 0:
        return np.float32(bce)

    p64 = p.astype(np.float64)
    s64 = ss.astype(np.float64)
    A = _dist_rowsums(p64)
    B = _dist_rowsums(s64)
    sAB = A @ B
    sAA = A @ A
    sBB = B @ B
    Tx = A.sum()
    Ty = B.sum()
    smp = p64.sum()
    smp2 = (p64 * p64).sum()
    sms = s64.sum()
    sms2 = (s64 * s64).sum()
    Sxx = 2.0 * c * smp2 - 2.0 * smp * smp
    Syy = 2.0 * c * sms2 - 2.0 * sms * sms
    Vxy = Sxy - (2.0 / c) * sAB + Tx * Ty / (c * c)
    Vxx = Sxx - (2.0 / c) * sAA + Tx * Tx / (c * c)
    Vyy = Syy - (2.0 / c) * sBB + Ty * Ty / (c * c)
    dcov = np.sqrt(max(Vxy / (c * c), EPS))
    dvx = np.sqrt(max(Vxx / (c * c), EPS))
    dvy = np.sqrt(max(Vyy / (c * c), EPS))
    dcor = dcov / (dvx * dvy)
    return np.float32(bce + dcor)


def _prepare(inputs, targets, spectators):
    x = np.ascontiguousarray(np.asarray(inputs, dtype=np.float32)).reshape(N)
    t = np.ascontiguousarray(np.asarray(targets, dtype=np.float32)).reshape(N)
    s = np.ascontiguousarray(np.asarray(spectators, dtype=np.float32)).reshape(N)
    idx = np.nonzero(s >= 0.5)[0]
    c = len(idx)
    cap = 4096 if c <= 4096 else 8192
    xs64 = x[idx].astype(np.float64)
    p = (1.0 / (1.0 + np.exp(-xs64))).astype(np.float32)
    ss = s[idx]
    in_maps = _make_in_maps(x, t, p, ss, c, cap)
    return _get_nc(cap), in_maps, (p, ss, c, cap)


def kernel(inputs, targets, spectators):
    nc, in_maps, meta = _prepare(inputs, targets, spectators)
    res = run_bass_kernel_spmd(nc, in_maps, list(range(NCORES)))
    return _combine(res.results, *meta)


# revision 19
# speedup vs baseline: 6.9715x; 1.0942x over previous
"""Trainium2 Bass kernel for nn_ConditionalDisCoLoss.

loss = BCEWithLogits(inputs, targets)
     + dCor_masked(sigmoid(inputs), spectators, mask=spectators>=0.5)

Key identities (see _combine):
  With A_i = sum_j m_j|p_i-p_j|, B_i likewise for s, and
  Sxy = sum_ij m_i m_j |p_i-p_j||s_i-s_j|:
    Vxy = Sxy - (2/c) sum A_i B_i + (sum A)(sum B)/c^2   (and Vxx, Vyy)
  A_i, B_i have O(n log n) closed forms via sorting (1-D data), and
  Sxx, Syy have O(n) closed forms, so the ONLY O(n^2) quantity is Sxy.

Device computes Sxy and the BCE partials; host does the O(n)/O(n log n)
filtering, packing and scalar assembly.

Sxy device trick: |a*b| == |a|*|b| exactly in IEEE, and
  D1*D2 = m_i m_j (p_i-p_j)(s_i-s_j)
        = (m_i p_i s_i)*m_j - (m_i p_i)*(m_j s_j) - (m_i s_i)*(m_j p_j)
          + m_i*(m_j p_j s_j)
is a rank-4 bilinear form -> ONE K=4 f32r matmul produces D1*D2 directly
in PSUM; a single elementwise op with fused row-accumulation per
[128 x 1024] tile yields the Sxy partials.  The per-tile op alternates
between ACT (native Abs) and DVE; DVE has no 1-op abs-from-PSUM, so it
accumulates relu(T) and the host applies sum|T| = 2*sum relu(T) - sum T
with sum T in closed form (rank-4 again -> O(1) per tile).

BCE avoids the Ln activation table entirely (Abs+Exp share one table,
so the single LoadActFuncSet runs during the DMA window):
ln(1+v) for v=exp(-|x|) in (0,1] is a degree-7 polynomial on DVE/Pool
(max err 5.7e-7).

Distribution: samples with m=1 are host-compacted (c ~ n/2) and padded
to CAP=4096 (pad rows get m=0 and drop out).  32 global row-tiles of
128 are dealt round-robin: core k owns i-tiles {8t+k}, whose 1024-wide
band is t, so every core runs the SAME program (jt in [it, NB)) - 10
tiles each.  Diagonal-band tiles cover their band block fully (counted
once); upper tiles are doubled in the combine.  BCE runs on contiguous
1024-row strips of the raw inputs.  Falls back to a CAP=8192 build if
c > 4096.
"""

import numpy as np
from contextlib import ExitStack

import concourse.bass as bass
import concourse.bacc as bacc
import concourse.tile as tile
from concourse import mybir
from concourse.bass_utils import run_bass_kernel_spmd

N = 8192
NCORES = 8
P = 128
JT = 1024
BSTRIP = N // NCORES     # 1024 BCE rows per core
BF = BSTRIP // P         # 8 BCE cols
EPS = 1e-8

F32 = mybir.dt.float32
BF16 = mybir.dt.bfloat16
F32R = mybir.dt.float32r
ALU = mybir.AluOpType
ACTF = mybir.ActivationFunctionType
AX = mybir.AxisListType

# ln(1+v) on [0,1], degree-7 Chebyshev fit (max abs err 5.7e-7)
LN1P = (5.62195900721818e-07, 0.9999574870750696, -0.4992065685478763,
        0.32697310001391783, -0.2228362583278401, 0.13076503250360005,
        -0.05262485136716543, 0.010119082927575069)


def _tile_layout(nb):
    """Loop-order tile list [(it, jt, slot, is_act)] shared by _build and
    _combine.  Slots: diag tile (it,it) -> it; upper -> nb+u."""
    out = []
    ei = 0
    u = 0
    for it in range(nb):
        for jt in range(it, nb):
            sl = it if jt == it else nb + u
            if jt > it:
                u += 1
            out.append((it, jt, sl, ei % 2 == 0))
            ei += 1
    return out


def _build(cap):
    """cap: padded compacted-sample capacity (multiple of 1024, /8 cores)."""
    nb = cap // JT           # bands == i-tiles per core
    nit = nb
    ntiles = nb * (nb + 1) // 2
    rw = ntiles + 1          # Rp columns: tiles + bce
    opw = cap + P * nit      # packed operand width: [rhs | lhsT]

    nc = bacc.Bacc("TRN2", target_bir_lowering=False, debug=False,
                   num_devices=NCORES, enable_asserts=False)

    ops_d = nc.dram_tensor("ops4", [4, opw], F32, kind="ExternalInput")
    xs_d = nc.dram_tensor("x_strip", [BSTRIP], F32, kind="ExternalInput")
    ts_d = nc.dram_tensor("t_strip", [BSTRIP], F32, kind="ExternalInput")
    out_d = nc.dram_tensor("rowout", [P, rw], F32, kind="ExternalOutput")

    with tile.TileContext(nc) as tc, ExitStack() as ctx:
        pre = ctx.enter_context(tc.tile_pool(name="pre", bufs=1))
        uvp = ctx.enter_context(tc.tile_pool(name="uv", bufs=3))
        psp = ctx.enter_context(tc.tile_pool(name="psp", bufs=3, space="PSUM"))

        # act-table warmup: Abs+Exp live in one set ('exp_and_others');
        # issuing an Exp first loads that table during the DMA window.
        warm = pre.tile([P, 1], F32)
        nc.vector.memset(warm, 0.0)
        warm2 = pre.tile([P, 1], F32)
        nc.scalar.activation(warm2, warm, ACTF.Exp)

        # one HWDGE DMA for all matmul operands; BCE strip rides the
        # otherwise-idle software DGE on Pool.
        ops = pre.tile([4, opw], F32R)
        nc.sync.dma_start(out=ops, in_=ops_d.ap().bitcast(F32R))
        xs = pre.tile([P, BF], F32)
        nc.gpsimd.dma_start(out=xs, in_=xs_d.ap().rearrange("(p f) -> p f", p=P))
        ts = pre.tile([P, BF], F32)
        nc.gpsimd.dma_start(out=ts, in_=ts_d.ap().rearrange("(p f) -> p f", p=P))
        rhs = ops[:, 0:cap]
        lhsT = ops[:, cap:opw]

        Rp = pre.tile([P, rw], F32)

        # ---------- main pass ----------
        for it, jt, sl, is_act in _tile_layout(nb):
            lA = lhsT[:, it * P:(it + 1) * P]
            psT = psp.tile([P, JT], F32, tag="ps")
            for h in range(JT // 512):
                j0 = jt * JT + h * 512
                nc.tensor.matmul(psT[:, h * 512:(h + 1) * 512],
                                 lhsT=lA, rhs=rhs[:, j0:j0 + 512],
                                 start=True, stop=True)
            W = uvp.tile([P, JT], BF16, tag="W")
            if is_act:
                nc.scalar.activation(W, psT, ACTF.Abs,
                                     accum_out=Rp[:, sl:sl + 1])
            else:
                nc.vector.tensor_scalar(W, psT, 0.0, 0.0, ALU.max,
                                        ALU.add, accum_out=Rp[:, sl:sl + 1])

        # ---------- BCE partial: relu(x) - x*t + ln(1+exp(-|x|)) ----------
        ax = pre.tile([P, BF], F32)
        nc.vector.scalar_tensor_tensor(out=ax, in0=xs, scalar=-1.0, in1=xs,
                                       op0=ALU.mult, op1=ALU.max)
        v = pre.tile([P, BF], F32)
        nc.scalar.activation(v, ax, ACTF.Exp, scale=-1.0)

        # ln(1+v) via degree-7 Estrin on DVE (pairs) + Pool (products/sums)
        c = LN1P
        p01 = pre.tile([P, BF], F32)
        nc.vector.tensor_scalar(p01, v, c[1], c[0], ALU.mult, ALU.add)
        p23 = pre.tile([P, BF], F32)
        nc.vector.tensor_scalar(p23, v, c[3], c[2], ALU.mult, ALU.add)
        p45 = pre.tile([P, BF], F32)
        nc.vector.tensor_scalar(p45, v, c[5], c[4], ALU.mult, ALU.add)
        p67 = pre.tile([P, BF], F32)
        nc.vector.tensor_scalar(p67, v, c[7], c[6], ALU.mult, ALU.add)
        v2 = pre.tile([P, BF], F32)
        nc.gpsimd.tensor_tensor(out=v2, in0=v, in1=v, op=ALU.mult)
        v4 = pre.tile([P, BF], F32)
        nc.gpsimd.tensor_tensor(out=v4, in0=v2, in1=v2, op=ALU.mult)
        r1 = pre.tile([P, BF], F32)
        nc.gpsimd.tensor_tensor(out=r1, in0=p23, in1=v2, op=ALU.mult)
        q1 = pre.tile([P, BF], F32)
        nc.gpsimd.tensor_tensor(out=q1, in0=r1, in1=p01, op=ALU.add)
        r2 = pre.tile([P, BF], F32)
        nc.gpsimd.tensor_tensor(out=r2, in0=p67, in1=v2, op=ALU.mult)
        q2 = pre.tile([P, BF], F32)
        nc.gpsimd.tensor_tensor(out=q2, in0=r2, in1=p45, op=ALU.add)
        r3 = pre.tile([P, BF], F32)
        nc.gpsimd.tensor_tensor(out=r3, in0=q2, in1=v4, op=ALU.mult)
        sp = pre.tile([P, BF], F32)
        nc.gpsimd.tensor_tensor(out=sp, in0=r3, in1=q1, op=ALU.add)

        rx = pre.tile([P, BF], F32)
        nc.vector.tensor_scalar(rx, xs, 0.0, None, ALU.max)
        xtm = pre.tile([P, BF], F32)
        nc.gpsimd.tensor_tensor(out=xtm, in0=xs, in1=ts, op=ALU.mult)
        t1 = pre.tile([P, BF], F32)
        nc.vector.tensor_tensor(out=t1, in0=rx, in1=xtm, op=ALU.subtract)
        t2 = pre.tile([P, BF], F32)
        nc.vector.scalar_tensor_tensor(out=t2, in0=t1, scalar=0.0, in1=sp,
                                       op0=ALU.add, op1=ALU.add,
                                       accum_out=Rp[:, ntiles:ntiles + 1])

        nc.sync.dma_start(out=out_d.ap(), in_=Rp)

    nc.compile()
    return nc


_NC_CACHE = {}


def _get_nc(cap=4096):
    if cap not in _NC_CACHE:
        _NC_CACHE[cap] = _build(cap)
    return _NC_CACHE[cap]


def _make_in_maps(x, t, p, ss, c, cap):
    """x,t: raw [N] f32. p,ss: compacted selected samples (len c) f32."""
    nit = cap // JT
    opw = cap + P * nit
    rhs = np.zeros((4, cap), np.float32)
    rhs[0, :c] = 1.0
    rhs[1, :c] = ss
    rhs[2, :c] = p
    rhs[3, :c] = p * ss
    in_maps = []
    for k in range(NCORES):
        ops = np.empty((4, opw), np.float32)
        ops[:, 0:cap] = rhs
        for t_ in range(nit):
            g = 8 * t_ + k
            sl = slice(P * g, P * g + P)
            dl = slice(cap + P * t_, cap + P * t_ + P)
            ops[0, dl] = rhs[3, sl]
            ops[1, dl] = -rhs[2, sl]
            ops[2, dl] = -rhs[1, sl]
            ops[3, dl] = rhs[0, sl]
        in_maps.append({
            "ops4": ops,
            "x_strip": np.ascontiguousarray(x[BSTRIP * k:BSTRIP * (k + 1)]),
            "t_strip": np.ascontiguousarray(t[BSTRIP * k:BSTRIP * (k + 1)]),
        })
    return in_maps


def _dist_rowsums(v):
    """A_i = sum_j |v_i - v_j| in O(n log n) (f64)."""
    v = v.astype(np.float64)
    n = len(v)
    order = np.argsort(v, kind="stable")
    sv = v[order]
    cs = np.cumsum(sv)
    S = cs[-1] if n else 0.0
    r = np.arange(1, n + 1)
    Ar = sv * (2 * r - n) - 2.0 * cs + S
    A = np.empty(n)
    A[order] = Ar
    return A


def _combine(results, p, ss, c, cap):
    nb = cap // JT
    ntiles = nb * (nb + 1) // 2

    # closed-form block sums for the relu->abs correction on DVE tiles:
    # T_ij = sum_r lhsT[r,i] rhs[r,j]  =>  sum_{tile} T = sum_r L_r(k,it) S_r(jt)
    rhs = np.zeros((4, cap))
    rhs[0, :c] = 1.0
    rhs[1, :c] = ss.astype(np.float64)
    rhs[2, :c] = p.astype(np.float64)
    rhs[3, :c] = rhs[1, :c] * rhs[2, :c]
    S = rhs.reshape(4, nb, JT).sum(axis=2)             # [4, nb]
    G = rhs.reshape(4, cap // P, P).sum(axis=2)        # [4, n_chunks]
    sgn = np.array([1.0, -1.0, -1.0, 1.0])
    layout = _tile_layout(nb)

    Sxy = 0.0
    bce_sum = 0.0
    for k in range(NCORES):
        r = results[k]["rowout"].astype(np.float64)
        for it, jt, sl, is_act in layout:
            v = r[:, sl].sum()
            if not is_act:
                g = 8 * it + k
                # L_r = chunk sums of lhsT rows (mps, -mp, -ms, m)
                L = sgn * G[::-1, g]
                v = 2.0 * v - float(L @ S[:, jt])
            Sxy += v if jt == it else 2.0 * v
        bce_sum += r[:, ntiles].sum()
    bce = bce_sum / N
    if c == 0:
        return np.float32(bce)

    p64 = p.astype(np.float64)
    s64 = ss.astype(np.float64)
    A = _dist_rowsums(p64)
    B = _dist_rowsums(s64)
    sAB = A @ B
    sAA = A @ A
    sBB = B @ B
    Tx = A.sum()
    Ty = B.sum()
    smp = p64.sum()
    smp2 = (p64 * p64).sum()
    sms = s64.sum()
    sms2 = (s64 * s64).sum()
    Sxx = 2.0 * c * smp2 - 2.0 * smp * smp
    Syy = 2.0 * c * sms2 - 2.0 * sms * sms
    Vxy = Sxy - (2.0 / c) * sAB + Tx * Ty / (c * c)
    Vxx = Sxx - (2.0 / c) * sAA + Tx * Tx / (c * c)
    Vyy = Syy - (2.0 / c) * sBB + Ty * Ty / (c * c)
    dcov = np.sqrt(max(Vxy / (c * c), EPS))
    dvx = np.sqrt(max(Vxx / (c * c), EPS))
    dvy = np.sqrt(max(Vyy / (c * c), EPS))
    dcor = dcov / (dvx * dvy)
    return np.float32(bce + dcor)


def _prepare(inputs, targets, spectators):
    x = np.ascontiguousarray(np.asarray(inputs, dtype=np.float32)).reshape(N)
    t = np.ascontiguousarray(np.asarray(targets, dtype=np.float32)).reshape(N)
    s = np.ascontiguousarray(np.asarray(spectators, dtype=np.float32)).reshape(N)
    idx = np.nonzero(s >= 0.5)[0]
    c = len(idx)
    cap = 4096 if c <= 4096 else 8192
    xs64 = x[idx].astype(np.float64)
    p = (1.0 / (1.0 + np.exp(-xs64))).astype(np.float32)
    ss = s[idx]
    in_maps = _make_in_maps(x, t, p, ss, c, cap)
    return _get_nc(cap), in_maps, (p, ss, c, cap)


def kernel(inputs, targets, spectators):
    nc, in_maps, meta = _prepare(inputs, targets, spectators)
    res = run_bass_kernel_spmd(nc, in_maps, list(range(NCORES)))
    return _combine(res.results, *meta)


# revision 20
# speedup vs baseline: 7.1023x; 1.0188x over previous
"""Trainium2 Bass kernel for nn_ConditionalDisCoLoss.

loss = BCEWithLogits(inputs, targets)
     + dCor_masked(sigmoid(inputs), spectators, mask=spectators>=0.5)

Key identities (see _combine):
  With A_i = sum_j m_j|p_i-p_j|, B_i likewise for s, and
  Sxy = sum_ij m_i m_j |p_i-p_j||s_i-s_j|:
    Vxy = Sxy - (2/c) sum A_i B_i + (sum A)(sum B)/c^2   (and Vxx, Vyy)
  A_i, B_i have O(n log n) closed forms via sorting (1-D data), and
  Sxx, Syy have O(n) closed forms, so the ONLY O(n^2) quantity is Sxy.

Device computes Sxy and the BCE partials; host does the O(n)/O(n log n)
filtering, packing and scalar assembly.

Sxy device trick: |a*b| == |a|*|b| exactly in IEEE, and
  D1*D2 = m_i m_j (p_i-p_j)(s_i-s_j)
        = (m_i p_i s_i)*m_j - (m_i p_i)*(m_j s_j) - (m_i s_i)*(m_j p_j)
          + m_i*(m_j p_j s_j)
is a rank-4 bilinear form -> ONE K=4 f32r matmul produces D1*D2 directly
in PSUM; a single elementwise op with fused row-accumulation per
[128 x 1024] tile yields the Sxy partials.  The per-tile op alternates
between ACT (native Abs) and DVE; DVE has no 1-op abs-from-PSUM, so it
accumulates relu(T) and the host applies sum|T| = 2*sum relu(T) - sum T
with sum T in closed form (rank-4 again -> O(1) per tile).

BCE avoids the Ln activation table entirely (Abs+Exp share one table,
so the single LoadActFuncSet runs during the DMA window):
ln(1+v) for v=exp(-|x|) in (0,1] is a degree-7 polynomial on DVE/Pool
(max err 5.7e-7).

Distribution: samples with m=1 are host-compacted (c ~ n/2) and padded
to CAP=4096 (pad rows get m=0 and drop out).  32 global row-tiles of
128 are dealt round-robin: core k owns i-tiles {8t+k}, whose 1024-wide
band is t, so every core runs the SAME program (jt in [it, NB)) - 10
tiles each.  Diagonal-band tiles cover their band block fully (counted
once); upper tiles are doubled in the combine.  BCE runs on contiguous
1024-row strips of the raw inputs.  Falls back to a CAP=8192 build if
c > 4096.
"""

import numpy as np
from contextlib import ExitStack

import concourse.bass as bass
import concourse.bacc as bacc
import concourse.tile as tile
from concourse import mybir
from concourse.bass_utils import run_bass_kernel_spmd

N = 8192
NCORES = 8
P = 128
JT = 1024
BSTRIP = N // NCORES     # 1024 BCE rows per core
BF = BSTRIP // P         # 8 BCE cols
EPS = 1e-8

F32 = mybir.dt.float32
BF16 = mybir.dt.bfloat16
F32R = mybir.dt.float32r
ALU = mybir.AluOpType
ACTF = mybir.ActivationFunctionType
AX = mybir.AxisListType

# ln(1+v) on [0,1], degree-7 Chebyshev fit (max abs err 5.7e-7)
LN1P = (5.62195900721818e-07, 0.9999574870750696, -0.4992065685478763,
        0.32697310001391783, -0.2228362583278401, 0.13076503250360005,
        -0.05262485136716543, 0.010119082927575069)


def _tile_layout(nb):
    """Loop-order tile list [(it, jt, slot, is_act)] shared by _build and
    _combine.  Slots: diag tile (it,it) -> it; upper -> nb+u."""
    out = []
    ei = 0
    u = 0
    for it in range(nb):
        for jt in range(it, nb):
            sl = it if jt == it else nb + u
            if jt > it:
                u += 1
            out.append((it, jt, sl, ei % 2 == 0))
            ei += 1
    return out


def _build(cap):
    """cap: padded compacted-sample capacity (multiple of 1024, /8 cores)."""
    nb = cap // JT           # bands == i-tiles per core
    nit = nb
    ntiles = nb * (nb + 1) // 2
    rw = ntiles + 1          # Rp columns: tiles + bce
    opw = cap + P * nit      # packed operand width: [rhs | lhsT]

    nc = bacc.Bacc("TRN2", target_bir_lowering=False, debug=False,
                   num_devices=NCORES, enable_asserts=False)

    ops_d = nc.dram_tensor("ops4", [4, opw], F32, kind="ExternalInput")
    xs_d = nc.dram_tensor("x_strip", [BSTRIP], F32, kind="ExternalInput")
    ts_d = nc.dram_tensor("t_strip", [BSTRIP], F32, kind="ExternalInput")
    out_d = nc.dram_tensor("rowout", [P, rw], F32, kind="ExternalOutput")

    with tile.TileContext(nc) as tc, ExitStack() as ctx:
        pre = ctx.enter_context(tc.tile_pool(name="pre", bufs=1))
        uvp = ctx.enter_context(tc.tile_pool(name="uv", bufs=3))
        psp = ctx.enter_context(tc.tile_pool(name="psp", bufs=3, space="PSUM"))

        # act-table warmup: Abs+Exp live in one set ('exp_and_others');
        # issuing an Exp first loads that table during the DMA window.
        warm = pre.tile([P, 1], F32)
        nc.vector.memset(warm, 0.0)
        warm2 = pre.tile([P, 1], F32)
        nc.scalar.activation(warm2, warm, ACTF.Exp)

        # one HWDGE DMA for all matmul operands; BCE strip rides the
        # otherwise-idle software DGE on Pool.
        ops = pre.tile([4, opw], F32R)
        nc.sync.dma_start(out=ops, in_=ops_d.ap().bitcast(F32R))
        xs = pre.tile([P, BF], F32)
        nc.gpsimd.dma_start(out=xs, in_=xs_d.ap().rearrange("(p f) -> p f", p=P))
        ts = pre.tile([P, BF], F32)
        nc.gpsimd.dma_start(out=ts, in_=ts_d.ap().rearrange("(p f) -> p f", p=P))
        rhs = ops[:, 0:cap]
        lhsT = ops[:, cap:opw]

        Rp = pre.tile([P, rw], F32)

        # ---------- main pass ----------
        for it, jt, sl, is_act in _tile_layout(nb):
            lA = lhsT[:, it * P:(it + 1) * P]
            psT = psp.tile([P, JT], F32, tag="ps")
            for h in range(JT // 512):
                j0 = jt * JT + h * 512
                nc.tensor.matmul(psT[:, h * 512:(h + 1) * 512],
                                 lhsT=lA, rhs=rhs[:, j0:j0 + 512],
                                 start=True, stop=True)
            W = uvp.tile([P, JT], BF16, tag="W")
            if is_act:
                nc.scalar.activation(W, psT, ACTF.Abs,
                                     accum_out=Rp[:, sl:sl + 1])
            else:
                nc.vector.tensor_scalar(W, psT, 0.0, 0.0, ALU.max,
                                        ALU.add, accum_out=Rp[:, sl:sl + 1])

        # ---------- BCE partial: relu(x) - x*t + ln(1+exp(-|x|)) ----------
        # high_priority + a shallow DVE/Pool split keep the whole chain in
        # the startup window so it never blocks the big per-tile DVE ops.
        with tc.high_priority():
            ax = pre.tile([P, BF], F32)
            nc.vector.scalar_tensor_tensor(out=ax, in0=xs, scalar=-1.0,
                                           in1=xs, op0=ALU.mult, op1=ALU.max)
            v = pre.tile([P, BF], F32)
            nc.scalar.activation(v, ax, ACTF.Exp, scale=-1.0)

            # ln(1+v), degree-7 Estrin: pairs on DVE, chain on Pool
            c = LN1P
            p01 = pre.tile([P, BF], F32)
            nc.vector.tensor_scalar(p01, v, c[1], c[0], ALU.mult, ALU.add)
            p23 = pre.tile([P, BF], F32)
            nc.vector.tensor_scalar(p23, v, c[3], c[2], ALU.mult, ALU.add)
            p45 = pre.tile([P, BF], F32)
            nc.vector.tensor_scalar(p45, v, c[5], c[4], ALU.mult, ALU.add)
            p67 = pre.tile([P, BF], F32)
            nc.vector.tensor_scalar(p67, v, c[7], c[6], ALU.mult, ALU.add)
            v2 = pre.tile([P, BF], F32)
            nc.gpsimd.tensor_tensor(out=v2, in0=v, in1=v, op=ALU.mult)
            v4 = pre.tile([P, BF], F32)
            nc.gpsimd.tensor_tensor(out=v4, in0=v2, in1=v2, op=ALU.mult)
            r1 = pre.tile([P, BF], F32)
            nc.vector.tensor_tensor(out=r1, in0=p23, in1=v2, op=ALU.mult)
            r2 = pre.tile([P, BF], F32)
            nc.vector.tensor_tensor(out=r2, in0=p67, in1=v2, op=ALU.mult)
            q1 = pre.tile([P, BF], F32)
            nc.gpsimd.tensor_tensor(out=q1, in0=r1, in1=p01, op=ALU.add)
            q2 = pre.tile([P, BF], F32)
            nc.gpsimd.tensor_tensor(out=q2, in0=r2, in1=p45, op=ALU.add)
            r3 = pre.tile([P, BF], F32)
            nc.gpsimd.tensor_tensor(out=r3, in0=q2, in1=v4, op=ALU.mult)
            sp = pre.tile([P, BF], F32)
            nc.gpsimd.tensor_tensor(out=sp, in0=r3, in1=q1, op=ALU.add)

            rx = pre.tile([P, BF], F32)
            nc.vector.tensor_scalar(rx, xs, 0.0, None, ALU.max)
            xtm = pre.tile([P, BF], F32)
            nc.gpsimd.tensor_tensor(out=xtm, in0=xs, in1=ts, op=ALU.mult)
            t1 = pre.tile([P, BF], F32)
            nc.vector.tensor_tensor(out=t1, in0=rx, in1=xtm, op=ALU.subtract)
            t2 = pre.tile([P, BF], F32)
            nc.vector.scalar_tensor_tensor(out=t2, in0=t1, scalar=0.0,
                                           in1=sp, op0=ALU.add, op1=ALU.add,
                                           accum_out=Rp[:, ntiles:ntiles + 1])

        nc.sync.dma_start(out=out_d.ap(), in_=Rp)

    nc.compile()
    return nc


_NC_CACHE = {}


def _get_nc(cap=4096):
    if cap not in _NC_CACHE:
        _NC_CACHE[cap] = _build(cap)
    return _NC_CACHE[cap]


def _make_in_maps(x, t, p, ss, c, cap):
    """x,t: raw [N] f32. p,ss: compacted selected samples (len c) f32."""
    nit = cap // JT
    opw = cap + P * nit
    rhs = np.zeros((4, cap), np.float32)
    rhs[0, :c] = 1.0
    rhs[1, :c] = ss
    rhs[2, :c] = p
    rhs[3, :c] = p * ss
    in_maps = []
    for k in range(NCORES):
        ops = np.empty((4, opw), np.float32)
        ops[:, 0:cap] = rhs
        for t_ in range(nit):
            g = 8 * t_ + k
            sl = slice(P * g, P * g + P)
            dl = slice(cap + P * t_, cap + P * t_ + P)
            ops[0, dl] = rhs[3, sl]
            ops[1, dl] = -rhs[2, sl]
            ops[2, dl] = -rhs[1, sl]
            ops[3, dl] = rhs[0, sl]
        in_maps.append({
            "ops4": ops,
            "x_strip": np.ascontiguousarray(x[BSTRIP * k:BSTRIP * (k + 1)]),
            "t_strip": np.ascontiguousarray(t[BSTRIP * k:BSTRIP * (k + 1)]),
        })
    return in_maps


def _dist_rowsums(v):
    """A_i = sum_j |v_i - v_j| in O(n log n) (f64)."""
    v = v.astype(np.float64)
    n = len(v)
    order = np.argsort(v, kind="stable")
    sv = v[order]
    cs = np.cumsum(sv)
    S = cs[-1] if n else 0.0
    r = np.arange(1, n + 1)
    Ar = sv * (2 * r - n) - 2.0 * cs + S
    A = np.empty(n)
    A[order] = Ar
    return A


def _combine(results, p, ss, c, cap):
    nb = cap // JT
    ntiles = nb * (nb + 1) // 2

    # closed-form block sums for the relu->abs correction on DVE tiles:
    # T_ij = sum_r lhsT[r,i] rhs[r,j]  =>  sum_{tile} T = sum_r L_r(k,it) S_r(jt)
    rhs = np.zeros((4, cap))
    rhs[0, :c] = 1.0
    rhs[1, :c] = ss.astype(np.float64)
    rhs[2, :c] = p.astype(np.float64)
    rhs[3, :c] = rhs[1, :c] * rhs[2, :c]
    S = rhs.reshape(4, nb, JT).sum(axis=2)             # [4, nb]
    G = rhs.reshape(4, cap // P, P).sum(axis=2)        # [4, n_chunks]
    sgn = np.array([1.0, -1.0, -1.0, 1.0])
    layout = _tile_layout(nb)

    Sxy = 0.0
    bce_sum = 0.0
    for k in range(NCORES):
        r = results[k]["rowout"].astype(np.float64)
        for it, jt, sl, is_act in layout:
            v = r[:, sl].sum()
            if not is_act:
                g = 8 * it + k
                # L_r = chunk sums of lhsT rows (mps, -mp, -ms, m)
                L = sgn * G[::-1, g]
                v = 2.0 * v - float(L @ S[:, jt])
            Sxy += v if jt == it else 2.0 * v
        bce_sum += r[:, ntiles].sum()
    bce = bce_sum / N
    if c == 0:
        return np.float32(bce)

    p64 = p.astype(np.float64)
    s64 = ss.astype(np.float64)
    A = _dist_rowsums(p64)
    B = _dist_rowsums(s64)
    sAB = A @ B
    sAA = A @ A
    sBB = B @ B
    Tx = A.sum()
    Ty = B.sum()
    smp = p64.sum()
    smp2 = (p64 * p64).sum()
    sms = s64.sum()
    sms2 = (s64 * s64).sum()
    Sxx = 2.0 * c * smp2 - 2.0 * smp * smp
    Syy = 2.0 * c * sms2 - 2.0 * sms * sms
    Vxy = Sxy - (2.0 / c) * sAB + Tx * Ty / (c * c)
    Vxx = Sxx - (2.0 / c) * sAA + Tx * Tx / (c * c)
    Vyy = Syy - (2.0 / c) * sBB + Ty * Ty / (c * c)
    dcov = np.sqrt(max(Vxy / (c * c), EPS))
    dvx = np.sqrt(max(Vxx / (c * c), EPS))
    dvy = np.sqrt(max(Vyy / (c * c), EPS))
    dcor = dcov / (dvx * dvy)
    return np.float32(bce + dcor)


def _prepare(inputs, targets, spectators):
    x = np.ascontiguousarray(np.asarray(inputs, dtype=np.float32)).reshape(N)
    t = np.ascontiguousarray(np.asarray(targets, dtype=np.float32)).reshape(N)
    s = np.ascontiguousarray(np.asarray(spectators, dtype=np.float32)).reshape(N)
    idx = np.nonzero(s >= 0.5)[0]
    c = len(idx)
    cap = 4096 if c <= 4096 else 8192
    xs64 = x[idx].astype(np.float64)
    p = (1.0 / (1.0 + np.exp(-xs64))).astype(np.float32)
    ss = s[idx]
    in_maps = _make_in_maps(x, t, p, ss, c, cap)
    return _get_nc(cap), in_maps, (p, ss, c, cap)


def kernel(inputs, targets, spectators):
    nc, in_maps, meta = _prepare(inputs, targets, spectators)
    res = run_bass_kernel_spmd(nc, in_maps, list(range(NCORES)))
    return _combine(res.results, *meta)
